# revision 8
# baseline (speedup 1.0000x reference)
"""Trainium2 Bass kernel for the ChemotaxisPINN loss (v2, optimized).

loss = mean_col((u_t - D*u_xx + chi*(u_x*S'(x) + u*S''(x)))^2)
     + mean_ic((u - ic(x))^2) + mean_bc(u_x(0,t)^2) + mean_bc(u_x(1,t)^2)
for a 5-layer SiLU MLP u(x,t), via forward-mode AD on device.

Data-parallel over 8 cores; per core 17408 points = 34 chunks x 512, in a
feature-major [128 x points] layout.

Key optimizations over v1:
- Streams are fp16 (value/x-tangent/d-tangent) and float32r (xx-stream), so
  every matmul runs at 1 cycle/row on the PE (4x over fp32).
- The t-tangent is folded into a per-point *coefficient-seeded* direction
  d = (C1(x), A): u_d = C1*u_x + A*u_t directly, removing one stream.
- silu'(z) and silu''(z) are computed by custom fused DVE ops (one
  instruction each) from sigmoid s and y=z+b produced on the ACT engine.
- Elementwise work is spread over ACT / DVE / GPSIMD.

Residual per point: r = u_d + B*u_xx + C2*u - T, loss = sum r^2.
"""

import numpy as np

import orjson
import concourse.bass as bass
import concourse.tile as tile
from concourse import mybir
from concourse import dve_ops as DOPS
from concourse.dve_spec import Spec, Src0, Src1, C2, One, lower, _has_src1
from concourse.dve_uop import DveOpSpec
from concourse.bass_utils import run_bass_kernel_spmd

F32 = mybir.dt.float32
F32R = mybir.dt.float32r
F16 = mybir.dt.float16
AL = mybir.AluOpType
AF = mybir.ActivationFunctionType

N_CORES = 8
H = 128
N_COL, N_IC, N_BC = 131072, 2000, 2000
COLC = N_COL // N_CORES     # 16384
ICC = N_IC // N_CORES       # 250
BCC = N_BC // N_CORES       # 250
NVALID = COLC + ICC + 2 * BCC   # 17134
CH = 512                     # points per chunk
NCHUNK = (NVALID + CH - 1) // CH  # 34
NPTS = NCHUNK * CH           # 17408
F2 = NPTS // 128             # 136 (phase-2 free dim)

# ---------------------------------------------------------------------------
# BIR fix: this walrus build accepts at most ONE sem wait per instruction,
# while Tile attaches several.  Split extras onto single-wait NoOps.
# ---------------------------------------------------------------------------
_orig_to_json_bytes = bass.Bass.to_json_bytes


def _split_multiwait(m):
    for fn in m.get("functions", []):
        for blk in fn.get("blocks", []):
            insts = blk.get("instructions", [])
            out = []
            changed = False
            ctr = 0
            for inst in insts:
                si = inst.get("sync_info")
                waits = (si or {}).get("on_wait") or []
                if len(waits) > 1:
                    changed = True
                    for w in waits[:-1]:
                        ctr += 1
                        out.append({
                            "engine": inst["engine"],
                            "ins": [],
                            "outs": [],
                            "name": f"I-mws-{ctr}-{inst.get('name', '')}",
                            "opcode": "NoOp",
                            "sync_info": {"on_wait": [w], "on_update": []},
                            "debug": inst.get("debug", 0),
                        })
                    si["on_wait"] = waits[-1:]
                out.append(inst)
            if changed:
                blk["instructions"] = out
    return m


def _patched_to_json_bytes(self):
    return orjson.dumps(_split_multiwait(orjson.loads(_orig_to_json_bytes(self))))


bass.Bass.to_json_bytes = _patched_to_json_bytes


# ---------------------------------------------------------------------------
# Custom fused DVE ops: silu'(y) and silu''(y) from (y, s=sigmoid(y)).
#   silu'  = s*(1 + y*(1-s))
#   silu'' = s*(1-s)*(2 + y*(1-2s))
# ---------------------------------------------------------------------------
def _register_dve_op(name, spec):
    for op in DOPS.OPS:
        if op.name == name:
            return op
    row = DOPS._CUSTOM_DVE_ROW_BASE + len(DOPS.OPS)
    assert row < 0x20, "custom DVE op row overflow"
    shas = {}
    for ver in ("v3", "v4"):
        tmp = DveOpSpec(name=name, opcode=row, uops=lower(spec, ver=ver),
                        rd1_en=_has_src1(spec))
        shas[ver] = tmp.sha(ver)
    op = DOPS.DveOp(name, spec, subdim=False, uops_sha=shas)
    DOPS.OPS.append(op)
    DOPS.CUSTOM_DVE_SPECS[name] = spec
    DOPS._SUB_OPCODE_FOR_NAME[name] = row
    return op


def _mk_silu_sp():
    u = Src0 * Src1
    v = Src0 - u
    w = v + One
    return Spec(
        body=w * Src1,
        reference=lambda in0, in1, s0, s1, imm2: (
            (in0.astype(np.float32) - in0 * in1 + 1.0) * in1
        ),
    )


def _mk_silu_spp():
    t1 = One - Src1
    p = Src1 * t1
    t3 = t1 - Src1
    t4 = Src0 * t3
    t5 = t4 + C2
    return Spec(
        body=p * t5,
        reference=lambda in0, in1, s0, s1, imm2: (
            (in1.astype(np.float32) * (1.0 - in1))
            * (in0 * (1.0 - 2.0 * in1) + imm2)
        ),
    )


SILU_SP = _register_dve_op("SILU_SP_PINN", _mk_silu_sp())
SILU_SPP = _register_dve_op("SILU_SPP_PINN", _mk_silu_spp())


def _mk_silu_sp2():
    m = Src0 * Src1
    u = Src0 - m
    return Spec(
        body=u + Src1,
        reference=lambda in0, in1, s0, s1, imm2: (
            in0.astype(np.float32) - in0 * in1 + in1
        ),
    )


SILU_SP2 = _register_dve_op("SILU_SP2_PINN", _mk_silu_sp2())


# ---------------------------------------------------------------------------
# Device program
# ---------------------------------------------------------------------------
def build_program():
    nc = bass.Bass("TRN2", target_bir_lowering=False, debug=False)

    xt_in = nc.declare_dram_parameter("xt", [2, NPTS], F16, isOutput=False)
    cd_in = nc.declare_dram_parameter("cd", [2, NPTS], F16, isOutput=False)
    coef_in = nc.declare_dram_parameter("coef", [3, NPTS], F32, isOutput=False)
    w0_in = nc.declare_dram_parameter("w0", [2, H], F16, isOutput=False)
    w123_in = nc.declare_dram_parameter("w123", [3, H, H], F32, isOutput=False)
    w4_in = nc.declare_dram_parameter("w4", [H, 1], F32, isOutput=False)
    pc_in = nc.declare_dram_parameter("pc", [H, 8], F32, isOutput=False)
    part_out = nc.declare_dram_parameter("partial", [1, 1], F32, isOutput=True)

    # DRAM staging for per-point u, ud, uxx (chunk-major)
    stage = [nc.dram_tensor(f"stage{i}", [NCHUNK, CH], F32) for i in range(3)]

    with tile.TileContext(nc) as tc:
        with (
            tc.tile_pool(name="consts", bufs=1) as cn,
            tc.tile_pool(name="sb", bufs=3) as sb,
            tc.tile_pool(name="out1", bufs=8) as out1,
            tc.tile_pool(name="ph2", bufs=1) as ph2,
            tc.tile_pool(name="ps", bufs=4, space="PSUM") as ps,
        ):
            # ---- constants ----
            w0_sb = cn.tile([2, H], F16)
            nc.sync.dma_start(w0_sb[:], w0_in[:])
            w123f = cn.tile([H, 3, H], F32)
            for l in range(3):
                nc.sync.dma_start(w123f[:, l, :], w123_in[l])
            w4f = cn.tile([H, 1], F32)
            nc.sync.dma_start(w4f[:], w4_in[:])
            pc_sb = cn.tile([H, 8], F32)
            nc.sync.dma_start(pc_sb[:], pc_in[:])
            # weight copies: fp16 for value/x/d streams, f32r for xx stream
            w123h = cn.tile([H, 3, H], F16)
            nc.vector.tensor_copy(w123h[:], w123f[:])
            w123r = cn.tile([H, 3, H], F32R)
            nc.vector.tensor_copy(w123r[:], w123f[:])
            w4h = cn.tile([H, 1], F16)
            nc.vector.tensor_copy(w4h[:], w4f[:])
            w4r = cn.tile([H, 1], F32R)
            nc.vector.tensor_copy(w4r[:], w4f[:])
            w123hs = cn.tile([H, 3, H], F16)
            nc.scalar.activation(w123hs[:], w123f[:], AF.Copy, scale=2.0 ** 12)
            w4hs = cn.tile([H, 1], F16)
            nc.scalar.activation(w4hs[:], w4f[:], AF.Copy, scale=2.0 ** 12)
            ones_sb = cn.tile([H, 1], F32)
            nc.vector.memset(ones_sb[:], 1.0)
            # pc columns: 0..3 = b0..b3, 4 = W0[0], 5 = W0[0]^2, 6.. = b/2
            b_ap = [pc_sb[:, i:i + 1] for i in range(4)]
            w0x_ap = pc_sb[:, 4:5]
            w0x2_ap = pc_sb[:, 5:6]
            bh_sb = cn.tile([H, 4], F32)
            nc.scalar.activation(bh_sb[:], pc_sb[:, 0:4], AF.Copy, scale=0.5)
            bh_ap = [bh_sb[:, i:i + 1] for i in range(4)]

            # ---- phase 1: per-chunk MLP + tangents (chunk pairs interleaved
            # to give the scheduler independent work to hide latency) ----
            st = {}

            def emit_l0(c):
                sl = slice(c * CH, (c + 1) * CH)
                xt_sb = sb.tile([2, CH], F16, tag="xt", bufs=6, name="xt_sb")
                nc.sync.dma_start(xt_sb[:], xt_in[:, sl])
                cd_sb = sb.tile([2, CH], F16, tag="cd", bufs=6, name="cd_sb")
                nc.sync.dma_start(cd_sb[:], cd_in[:, sl])

                z = ps.tile([H, CH], F32, tag="z", name="z")
                nc.tensor.matmul(z[:], w0_sb[:], xt_sb[:], start=True, stop=True)
                zd = ps.tile([H, CH], F32, tag="z", name="zd")
                nc.tensor.matmul(zd[:], w0_sb[:], cd_sb[:], start=True, stop=True)

                s_t = sb.tile([H, CH], F16, tag="s", bufs=6, name="s_t")
                nc.scalar.activation(s_t[:], z[:], AF.Sigmoid, bias=b_ap[0])
                y_t = sb.tile([H, CH], F16, tag="y", bufs=6, name="y_t")
                nc.scalar.activation(y_t[:], z[:], AF.Identity, bias=b_ap[0])
                th_t = sb.tile([H, CH], F16, tag="th", bufs=6, name="th_t")
                nc.scalar.activation(th_t[:], z[:], AF.Tanh, bias=bh_ap[0],
                                     scale=0.5)

                a_t = sb.tile([H, CH], F16, tag="a", bufs=6, name="a_t")
                nc.vector.tensor_tensor(a_t[:], y_t[:], s_t[:], AL.mult)
                sp_t = sb.tile([H, CH], F16, tag="sp", bufs=6, name="sp_t")
                nc.vector._custom_dve(SILU_SP, out=sp_t[:], in0=y_t[:], in1=s_t[:])
                q2_t = sb.tile([H, CH], F16, tag="q2", bufs=6, name="q2_t")
                nc.vector.tensor_tensor(q2_t[:], th_t[:], sp_t[:], AL.mult)
                spp_t = sb.tile([H, CH], F16, tag="spp", bufs=6, name="spp_t")
                nc.gpsimd.tensor_tensor(spp_t[:], s_t[:], q2_t[:], AL.subtract)

                ax_t = sb.tile([H, CH], F16, tag="ax", bufs=6, name="ax_t")
                nc.vector.tensor_scalar(ax_t[:], sp_t[:], w0x_ap, None, AL.mult)
                ad_t = sb.tile([H, CH], F16, tag="ad", bufs=6, name="ad_t")
                nc.vector.tensor_tensor(ad_t[:], sp_t[:], zd[:], AL.mult)
                xx_t = sb.tile([H, CH], F32R, tag="xx", bufs=6, name="xx_t")
                nc.vector.tensor_scalar(xx_t[:], spp_t[:], w0x2_ap, None, AL.mult)
                st[c] = (a_t, ax_t[:], ad_t[:], xx_t, None)

            def emit_layer(c, l):
                a_t, ax_t, ad_t, xx_t, xx2_t = st[c]
                Wh = w123h[:, l, :]
                Wr = w123r[:, l, :]
                z = ps.tile([H, CH], F32, tag="z", name="z")
                nc.tensor.matmul(z[:], Wh, a_t[:], start=True, stop=True)
                zxd = ps.tile([H, 2, CH], F32, tag="z2", bufs=2, name="zxd")
                nc.tensor.matmul(zxd[:, 0, :], Wh, ax_t, start=True, stop=True)
                nc.tensor.matmul(zxd[:, 1, :], Wh, ad_t, start=True, stop=True)
                zxx = ps.tile([H, CH], F32, tag="z", name="zxx")
                if l == 0:
                    nc.tensor.matmul(zxx[:], Wr, xx_t[:], start=True, stop=True)
                else:
                    nc.tensor.matmul(zxx[:], Wr, xx_t[:], start=True, stop=False)
                    nc.tensor.matmul(zxx[:], w123hs[:, l, :], xx2_t[:],
                                     start=False, stop=True)

                b = b_ap[l + 1]
                s_t = sb.tile([H, CH], F16, tag="s", bufs=6, name="s_t")
                nc.scalar.activation(s_t[:], z[:], AF.Sigmoid, bias=b)
                y_t = sb.tile([H, CH], F16, tag="y", bufs=6, name="y_t")
                nc.scalar.activation(y_t[:], z[:], AF.Identity, bias=b)
                th_t = sb.tile([H, CH], F16, tag="th", bufs=6, name="th_t")
                nc.scalar.activation(th_t[:], z[:], AF.Tanh, bias=bh_ap[l + 1],
                                     scale=0.5)
                zx2_t = sb.tile([H, CH], F16, tag="zx2", bufs=6, name="zx2_t")
                nc.scalar.activation(zx2_t[:], zxd[:, 0, :], AF.Square,
                                     scale=2.0 ** -6)

                a_t = sb.tile([H, CH], F16, tag="a", bufs=6, name="a_t")
                nc.vector.tensor_tensor(a_t[:], y_t[:], s_t[:], AL.mult)
                sp_t = sb.tile([H, CH], F16, tag="sp", bufs=6, name="sp_t")
                nc.vector._custom_dve(SILU_SP, out=sp_t[:], in0=y_t[:], in1=s_t[:])
                q2_t = sb.tile([H, CH], F16, tag="q2", bufs=6, name="q2_t")
                nc.vector.tensor_tensor(q2_t[:], th_t[:], sp_t[:], AL.mult)
                spp_t = sb.tile([H, CH], F16, tag="spp", bufs=6, name="spp_t")
                nc.gpsimd.tensor_tensor(spp_t[:], s_t[:], q2_t[:], AL.subtract)

                axd = sb.tile([H, 2, CH], F16, tag="axd", bufs=6, name="axd")
                sp_b = sp_t[:].rearrange("p (r f) -> p r f", r=1).broadcast_to(
                    (H, 2, CH))
                nc.vector.tensor_tensor(axd[:], zxd[:], sp_b, AL.mult)
                ax_t = axd[:, 0, :]
                ad_t = axd[:, 1, :]
                xx_t = sb.tile([H, CH], F32R, tag="xx", bufs=6, name="xx_t")
                nc.vector.tensor_tensor(xx_t[:], sp_t[:], zxx[:], AL.mult)
                xx2_t = sb.tile([H, CH], F16, tag="xx2", bufs=6, name="xx2_t")
                nc.vector.tensor_tensor(xx2_t[:], spp_t[:], zx2_t[:], AL.mult)
                st[c] = (a_t, ax_t, ad_t, xx_t, xx2_t)

            def emit_l4(c):
                a_t, ax_t, ad_t, xx_t, xx2_t = st[c]
                up = ps.tile([1, CH], F32, tag="z", name="up")
                nc.tensor.matmul(up[:], w4h[:], a_t[:], start=True, stop=True)
                u_sb = out1.tile([1, CH], F32, tag="u0", name="u_sb")
                nc.scalar.activation(u_sb[:], up[:], AF.Copy)
                nc.sync.dma_start(stage[0][c:c + 1, :], u_sb[:])

                udp = ps.tile([1, CH], F32, tag="z", name="udp")
                nc.tensor.matmul(udp[:], w4h[:], ad_t, start=True, stop=True)
                ud_sb = out1.tile([1, CH], F32, tag="u1", name="ud_sb")
                nc.scalar.activation(ud_sb[:], udp[:], AF.Copy)
                nc.sync.dma_start(stage[1][c:c + 1, :], ud_sb[:])

                uxp = ps.tile([1, CH], F32, tag="z", name="uxp")
                nc.tensor.matmul(uxp[:], w4r[:], xx_t[:], start=True, stop=False)
                nc.tensor.matmul(uxp[:], w4hs[:], xx2_t[:], start=False, stop=True)
                uxx_sb = out1.tile([1, CH], F32, tag="u2", name="uxx_sb")
                nc.scalar.activation(uxx_sb[:], uxp[:], AF.Copy)
                nc.sync.dma_start(stage[2][c:c + 1, :], uxx_sb[:])
                del st[c]

            for c0 in range(0, NCHUNK, 2):
                pair = [c0] if c0 + 1 >= NCHUNK else [c0, c0 + 1]
                for c in pair:
                    emit_l0(c)
                for l in range(3):
                    for c in pair:
                        emit_layer(c, l)
                for c in pair:
                    emit_l4(c)

            # ---- phase 2: residual + reduction ----
            sv = []
            for i in range(3):
                t = ph2.tile([128, F2], F32, tag=f"pu{i}")
                nc.sync.dma_start(
                    t[:],
                    stage[i].rearrange("a b -> (a b)").rearrange(
                        "(p f) -> p f", p=128))
                sv.append(t)
            u_v, ud_v, uxx_v = sv
            cf = []
            for k in range(3):
                t = ph2.tile([128, F2], F32, tag=f"pcf{k}")
                nc.sync.dma_start(t[:], coef_in[k].rearrange("(p f) -> p f", p=128))
                cf.append(t)
            cB, cC2, cT = cf

            r = ph2.tile([128, F2], F32)
            m = ph2.tile([128, F2], F32)
            nc.vector.tensor_tensor(m[:], uxx_v[:], cB[:], AL.mult)
            nc.vector.tensor_tensor(r[:], ud_v[:], m[:], AL.add)
            nc.vector.tensor_tensor(m[:], u_v[:], cC2[:], AL.mult)
            nc.vector.tensor_tensor(r[:], r[:], m[:], AL.add)
            nc.vector.tensor_tensor(r[:], r[:], cT[:], AL.subtract)

            rsq = ph2.tile([128, F2], F32)
            racc = ph2.tile([128, 1], F32)
            nc.vector.scalar_tensor_tensor(rsq[:], r[:], 1.0, r[:], AL.mult,
                                           AL.mult, accum_out=racc[:])
            lps = ps.tile([1, 1], F32, tag="z")
            nc.tensor.matmul(lps[:], racc[:], ones_sb[:], start=True, stop=True)
            lsb = ph2.tile([1, 1], F32)
            nc.vector.tensor_copy(lsb[:], lps[:])
            nc.sync.dma_start(part_out[:], lsb[:])

    mybir.codegen_inst_isa_subclasses(nc)
    return nc


# ---------------------------------------------------------------------------
# Host-side sharding + coefficient prep
# ---------------------------------------------------------------------------
def _host_inputs(inputs):
    x_col = np.asarray(inputs["x_col"], np.float64).reshape(-1)
    t_col = np.asarray(inputs["t_col"], np.float64).reshape(-1)
    x_ic = np.asarray(inputs["x_ic"], np.float64).reshape(-1)
    t_ic = np.asarray(inputs["t_ic"], np.float64).reshape(-1)
    x_bl = np.asarray(inputs["x_bc_left"], np.float64).reshape(-1)
    x_br = np.asarray(inputs["x_bc_right"], np.float64).reshape(-1)
    t_bc = np.asarray(inputs["t_bc"], np.float64).reshape(-1)
    W0 = np.asarray(inputs["W0"], np.float32)
    W4 = np.asarray(inputs["W4"], np.float32)
    b4 = float(np.asarray(inputs["b4"]).reshape(-1)[0])
    D = float(np.asarray(inputs["D"]))
    chi = float(np.asarray(inputs["chi"]))

    def S(x):
        return np.exp(-((x - 0.7) ** 2) / 0.02)

    def Sp(x):
        return -(x - 0.7) / 0.01 * S(x)

    def Spp(x):
        return S(x) * (((x - 0.7) ** 2) / 1.0e-4 - 100.0)

    def icf(x):
        return 0.1 + 0.05 * np.exp(-((x - 0.3) ** 2) / 0.01)

    swc = (1.0 / N_COL) ** 0.5
    swi = (1.0 / N_IC) ** 0.5
    swb = (1.0 / N_BC) ** 0.5

    pc = np.zeros((H, 8), np.float32)
    for i, k in enumerate(("b0", "b1", "b2", "b3")):
        pc[:, i] = np.asarray(inputs[k], np.float32)
    pc[:, 4] = W0[0]
    pc[:, 5] = W0[0] ** 2
    w123 = np.stack([np.asarray(inputs[k], np.float32) for k in ("W1", "W2", "W3")])

    in_maps = []
    for c in range(N_CORES):
        xs = np.full(NPTS, 0.5, np.float64)
        ts = np.full(NPTS, 0.5, np.float64)
        A = np.zeros(NPTS, np.float64)    # du/dt coefficient (-> d-seed row 1)
        B = np.zeros(NPTS, np.float64)    # u_xx coefficient
        C1 = np.zeros(NPTS, np.float64)   # du/dx coefficient (-> d-seed row 0)
        C2 = np.zeros(NPTS, np.float64)   # u coefficient
        TG = np.zeros(NPTS, np.float64)   # target

        o = 0
        sl = slice(c * COLC, (c + 1) * COLC)
        xs[o:o + COLC] = x_col[sl]
        ts[o:o + COLC] = t_col[sl]
        A[o:o + COLC] = swc
        B[o:o + COLC] = -D * swc
        C1[o:o + COLC] = chi * Sp(x_col[sl]) * swc
        C2[o:o + COLC] = chi * Spp(x_col[sl]) * swc
        o += COLC
        sl = slice(c * ICC, (c + 1) * ICC)
        xs[o:o + ICC] = x_ic[sl]
        ts[o:o + ICC] = t_ic[sl]
        C2[o:o + ICC] = swi
        TG[o:o + ICC] = swi * icf(x_ic[sl])
        o += ICC
        sl = slice(c * BCC, (c + 1) * BCC)
        xs[o:o + BCC] = x_bl[sl]
        ts[o:o + BCC] = t_bc[sl]
        C1[o:o + BCC] = swb
        o += BCC
        xs[o:o + BCC] = x_br[sl]
        ts[o:o + BCC] = t_bc[sl]
        C1[o:o + BCC] = swb
        o += BCC

        TG = TG - C2 * b4  # fold the final-layer bias into the target
        xt = np.stack([xs, ts]).astype(np.float16)
        cd = np.stack([C1, A]).astype(np.float16)
        # account for fp16 rounding of the d-seed so the folded coefficients
        # stay exactly consistent (seed row is what the device actually uses)
        coef = np.stack([B, C2, TG]).astype(np.float32)
        in_maps.append({
            "xt": xt, "cd": cd, "coef": coef,
            "w0": np.asarray(inputs["W0"], np.float16),
            "w123": w123, "w4": W4, "pc": pc,
        })
    return in_maps


_CACHE = {}


def _get_nc():
    if "nc" not in _CACHE:
        _CACHE["nc"] = build_program()
    return _CACHE["nc"]


def run(inputs, trace=False):
    nc = _get_nc()
    in_maps = _host_inputs(inputs)
    res = run_bass_kernel_spmd(nc, in_maps, list(range(N_CORES)), trace=trace)
    total = 0.0
    for i in range(N_CORES):
        total += float(res.results[i]["partial"][0, 0])
    return np.float32(total), res


def kernel(**inputs):
    loss, _ = run(inputs)
    return np.asarray(loss, np.float32)
